# revision 8
# baseline (speedup 1.0000x reference)
"""MinGRU LM Trainium2 kernel (8 NeuronCores, SPMD, no cross-core comms).

Sharding: core k -> batch k//2, sequence half k%2. Each core processes a
2560-token frame (390-token left halo for scan warm-up, 2048 valid, 122
right halo) of its batch fully independently; the halo replaces all
communication (the scan's dependence on tokens >64 steps back underflows
to zero in fp32, so a warmed-up local scan is bit-compatible).

Device layout is feature-major (D on the 128 partitions, tokens on the free
dim). The minGRU recurrence is a native DVE tensor_tensor_scan per chunk.
Matmuls use bf16 operands with fp32 PSUM accumulation; the residual stream
stays fp32 in SBUF. Out-of-sequence neighbors of the conv are zeroed via a
2-column mask; the scan is forced to c=1, v=0 on out-of-sequence tokens by
accumulating -40*(1-mask) into the z-gate logits with a rank-1 matmul.
g(x) = max(x+0.5, sigmoid(x)) (exact identity for the reference's g).
"""

import contextlib
import ctypes
import sys
import types
from contextlib import ExitStack

import numpy as np

SO_PATH = "/opt/axon/libaxon_pjrt.so"


def _ntff_profile_via_ctypes(so_path):
    try:
        lib = ctypes.CDLL(so_path)
    except OSError:
        return None
    if not hasattr(lib, "axon_start_nrt_profile"):
        return None
    lib.axon_start_nrt_profile.argtypes = [
        ctypes.POINTER(ctypes.c_int64),
        ctypes.c_size_t,
    ]
    lib.axon_start_nrt_profile.restype = ctypes.c_int64
    lib.axon_stop_nrt_profile.argtypes = [ctypes.c_char_p]
    lib.axon_stop_nrt_profile.restype = ctypes.c_int64

    @contextlib.contextmanager
    def _hook(output_dir, device_ids):
        import jax

        jax.devices()
        if device_ids:
            ids = (ctypes.c_int64 * len(device_ids))(*device_ids)
            rc = lib.axon_start_nrt_profile(ids, len(device_ids))
        else:
            rc = lib.axon_start_nrt_profile(None, 0)
        if rc != 0:
            raise RuntimeError(f"axon_start_nrt_profile rc={rc}")
        try:
            yield
        finally:
            n = lib.axon_stop_nrt_profile(str(output_dir).encode())
            if n < 0:
                raise RuntimeError(f"axon_stop_nrt_profile rc={n}")
            print(f"profile: {n} file(s) -> {output_dir}", file=sys.stderr)

    return _hook


def _install_hookshim():
    if "antenv.axon_hooks" in sys.modules:
        return
    try:
        import antenv  # noqa: F401
    except ImportError:
        return
    mod = types.ModuleType("antenv.axon_hooks")
    state = {"hook": _ntff_profile_via_ctypes(SO_PATH)}
    mod.set_axon_ntff_profile_hook = lambda h: state.__setitem__("hook", h)
    mod.get_axon_ntff_profile_hook = lambda: state["hook"]
    sys.modules["antenv.axon_hooks"] = mod
    import antenv as _a

    _a.axon_hooks = mod


_install_hookshim()

import concourse.bass as bass  # noqa: E402
import concourse.tile as tile  # noqa: E402
from concourse import mybir  # noqa: E402
from concourse.masks import make_identity  # noqa: E402

AF = mybir.ActivationFunctionType
OP = mybir.AluOpType
F32 = mybir.dt.float32
BF16 = mybir.dt.bfloat16
I32 = mybir.dt.int32

_spill_counter = [0]


def cap_sync_waits(nc, cap=1):
    """This walrus build rejects instructions with >1 semaphore wait. Spill
    excess waits onto same-engine NoOps inserted just before (the NX executes
    them in program order, so this is semantically identical)."""
    import bass_rust

    n_split = 0
    for f in nc.m.functions:
        for bb in f.blocks:
            insts = bb.instructions
            changed = False
            out = []
            for inst in insts:
                si = inst.sync_info
                if si is not None and len(si.on_wait) > cap:
                    waits = list(si.on_wait)
                    spill, keep = waits[:-cap], waits[-cap:]
                    for i in range(0, len(spill), cap):
                        chunk = spill[i : i + cap]
                        _spill_counter[0] += 1
                        nop = mybir.InstNoOp(
                            name=f"waitspill-{_spill_counter[0]}",
                            engine=inst.engine,
                        )
                        nop.sync_info = bass_rust.SyncInfo(
                            on_wait=chunk, on_update=[]
                        )
                        nc.register_instruction(nop, overwrite=True)
                        out.append(nop)
                    inst.sync_info = bass_rust.SyncInfo(
                        on_wait=keep, on_update=list(si.on_update)
                    )
                    n_split += 1
                    changed = True
                out.append(inst)
            if changed:
                bb.instructions = out
    return n_split


V, D, H, L, K = 32000, 1024, 1024, 6, 3
B, S = 4, 4096
P = 128
ND = D // P  # 8 feature tiles
SHIFT = 65
FRAME = 2560
LHALO = 390
VALID = 2048
NTT = FRAME // P  # 20 token tiles (gather)
CHUNK = 512
EPS = 1.1920929e-07  # jnp.finfo(f32).eps
MASK_COLS = (LHALO - 1, LHALO + VALID)  # conv-boundary columns 389, 2438


def _chunks(s0):
    out = []
    c = s0
    while c < FRAME:
        ln = min(CHUNK, FRAME - c)
        out.append((c, ln))
        c += ln
    return out


def build_program():
    nc = bass.Bass()

    ids_d = nc.dram_tensor("ids", [FRAME], I32, kind="ExternalInput")
    m2_d = nc.dram_tensor("m2", [1, 2], F32, kind="ExternalInput")
    omm_d = nc.dram_tensor("omm", [1, FRAME], BF16, kind="ExternalInput")
    hprev_d = nc.dram_tensor("hprev", [D], F32, kind="ExternalInput")
    emb_d = nc.dram_tensor("emb", [V, D], F32, kind="ExternalInput")
    cw_d = nc.dram_tensor("cw", [L, D, K], F32, kind="ExternalInput")
    cb_d = nc.dram_tensor("cb", [L, D], F32, kind="ExternalInput")
    pwT_d = nc.dram_tensor("pwT", [L, D, D], F32, kind="ExternalInput")
    pwb_d = nc.dram_tensor("pwb", [L, D], F32, kind="ExternalInput")
    n1w_d = nc.dram_tensor("n1w", [L, D], F32, kind="ExternalInput")
    n2w_d = nc.dram_tensor("n2w", [L, D], F32, kind="ExternalInput")
    zwT_d = nc.dram_tensor("zwT", [L, D, D], F32, kind="ExternalInput")
    zb_d = nc.dram_tensor("zb", [L, D], F32, kind="ExternalInput")
    hwT_d = nc.dram_tensor("hwT", [L, D, D], F32, kind="ExternalInput")
    hb_d = nc.dram_tensor("hb", [L, D], F32, kind="ExternalInput")
    f1w_d = nc.dram_tensor("f1w", [L, D, H], F32, kind="ExternalInput")
    f1b_d = nc.dram_tensor("f1b", [L, H], F32, kind="ExternalInput")
    f2w_d = nc.dram_tensor("f2w", [L, H, D], F32, kind="ExternalInput")
    f2b_d = nc.dram_tensor("f2b", [L, D], F32, kind="ExternalInput")
    nfw_d = nc.dram_tensor("nfw", [D], F32, kind="ExternalInput")
    outw_d = nc.dram_tensor("outw", [D, H], F32, kind="ExternalInput")
    outb_d = nc.dram_tensor("outb", [H], F32, kind="ExternalInput")
    y_d = nc.dram_tensor("y", [VALID, H], F32, kind="ExternalOutput")

    with tile.TileContext(nc) as tc, ExitStack() as ctx:
        persist = ctx.enter_context(tc.tile_pool(name="persist", bufs=1))
        wpool = ctx.enter_context(tc.tile_pool(name="wpool", bufs=2))
        wbpool = ctx.enter_context(tc.tile_pool(name="wbpool", bufs=2))
        apool = ctx.enter_context(tc.tile_pool(name="apool", bufs=1))
        bpool = ctx.enter_context(tc.tile_pool(name="bpool", bufs=1))
        spool = ctx.enter_context(tc.tile_pool(name="spool", bufs=2))
        gpool = ctx.enter_context(tc.tile_pool(name="gpool", bufs=2))
        lpool = ctx.enter_context(tc.tile_pool(name="lw", bufs=2))
        mm = ctx.enter_context(tc.tile_pool(name="mm", bufs=6, space="PSUM"))
        ssp = ctx.enter_context(tc.tile_pool(name="ssp", bufs=2, space="PSUM"))

        ones_col = persist.tile([P, 1], BF16)
        nc.vector.memset(ones_col[:], 1.0)
        one_row = persist.tile([1, P], F32)
        nc.vector.memset(one_row[:], 1.0)
        neg40 = persist.tile([1, P], BF16)
        nc.vector.memset(neg40[:], -40.0)

        # 2-column conv-boundary mask, broadcast across partitions via PE
        m2row = spool.tile([1, 2], F32, tag='tmprow', bufs=1)
        nc.sync.dma_start(out=m2row[:], in_=m2_d[:])
        m2ps = mm.tile([P, 2], F32, tag="ps")
        nc.tensor.matmul(out=m2ps[:], lhsT=one_row[:], rhs=m2row[:],
                         start=True, stop=True)
        mask2 = persist.tile([P, 2], F32)
        nc.vector.tensor_copy(mask2[:], m2ps[:])

        # 1-mask row (bf16) for the -40 z-gate trick
        orow_b = persist.tile([1, FRAME], BF16)
        nc.sync.dma_start(out=orow_b[:], in_=omm_d[:])

        # h_prev -> g(h_prev), laid out (P, ND)
        hp = persist.tile([P, ND], F32)
        nc.sync.dma_start(out=hp[:], in_=hprev_d.rearrange("(dt p) -> p dt", p=P))
        g_hp = persist.tile([P, ND], F32)
        sg_hp = spool.tile([P, ND], F32, tag="sghp")
        nc.scalar.activation(sg_hp[:], hp[:], AF.Sigmoid)
        lin_hp = spool.tile([P, ND], F32, tag="linhp")
        nc.vector.tensor_scalar_add(lin_hp[:], hp[:], 0.5)
        nc.vector.tensor_tensor(g_hp[:], lin_hp[:], sg_hp[:], op=OP.max)

        # out-bias broadcast (P, H)
        obrow = spool.tile([1, H], F32, tag='tmprow', bufs=1)
        nc.sync.dma_start(out=obrow[:], in_=outb_d[None, :])
        ob_bc = persist.tile([P, H], BF16)
        for oc in range(H // CHUNK):
            pb = mm.tile([P, CHUNK], F32, tag="ps")
            nc.tensor.matmul(out=pb[:], lhsT=one_row[:],
                             rhs=obrow[:, oc * CHUNK : (oc + 1) * CHUNK],
                             start=True, stop=True)
            nc.vector.tensor_copy(ob_bc[:, oc * CHUNK : (oc + 1) * CHUNK], pb[:])

        # residual stream: ND tiles (P, FRAME+2); cols 0 / FRAME+1 are zero
        # sentinels for the conv shifts
        hres = []
        for dt in range(ND):
            t = persist.tile([P, FRAME + 2], F32, tag=f"h{dt}")
            hres.append(t)
            nc.vector.memset(t[:, 0:1], 0.0)
            nc.vector.memset(t[:, FRAME + 1 : FRAME + 2], 0.0)

        # embedding gather (token-major) + PE transpose into hres
        ids_sb = persist.tile([P, NTT], I32)
        nc.sync.dma_start(out=ids_sb[:], in_=ids_d.rearrange("(tt p) -> p tt", p=P))
        ident = persist.tile([P, P], F32)
        make_identity(nc, ident[:])
        for tt in range(NTT):
            gath = wpool.tile([P, D], F32, tag="wraw")
            nc.gpsimd.indirect_dma_start(
                out=gath[:],
                out_offset=None,
                in_=emb_d[:],
                in_offset=bass.IndirectOffsetOnAxis(
                    ap=ids_sb[:, tt : tt + 1], axis=0
                ),
            )
            for dt in range(ND):
                pt = mm.tile([P, P], F32, tag="ps")
                nc.tensor.transpose(pt[:], gath[:, dt * P : (dt + 1) * P], ident[:])
                nc.vector.tensor_copy(
                    hres[dt][:, 1 + tt * P : 1 + (tt + 1) * P], pt[:]
                )

        def load_bias(dram_ap, tag):
            t = lpool.tile([P, ND], F32, tag=tag)
            nc.sync.dma_start(out=t[:], in_=dram_ap.rearrange("(dt p) -> p dt", p=P))
            return t

        def rmsnorm_to_bf16(s0, w_pp, xout):
            """xout[dt][:, c-s0] = bf16(h*w*rsqrt(mean_d(h^2)+eps)) over
            cols [s0, FRAME)."""
            for c0, ln in _chunks(s0):
                pss = ssp.tile([1, ln], F32, tag="pss")
                for dt in range(ND):
                    sq = spool.tile([P, ln], BF16, tag="sqc")
                    nc.scalar.activation(
                        sq[:], hres[dt][:, 1 + c0 : 1 + c0 + ln], AF.Square
                    )
                    nc.tensor.matmul(
                        out=pss[:], lhsT=ones_col[:], rhs=sq[:],
                        start=(dt == 0), stop=(dt == ND - 1),
                    )
                srow = spool.tile([1, ln], F32, tag="srow", bufs=1)
                nc.vector.tensor_scalar(
                    srow[:], pss[:], 1.0 / D, EPS, op0=OP.mult, op1=OP.add
                )
                rt = spool.tile([1, ln], F32, tag="rt", bufs=1)
                nc.scalar.activation(rt[:], srow[:], AF.Sqrt)
                nc.vector.reciprocal(srow[:], rt[:])
                pbc = mm.tile([P, ln], F32, tag="ps")
                nc.tensor.matmul(out=pbc[:], lhsT=one_row[:], rhs=srow[:],
                                 start=True, stop=True)
                for dt in range(ND):
                    nc.vector.scalar_tensor_tensor(
                        out=xout[dt][:, c0 - s0 : c0 - s0 + ln],
                        in0=hres[dt][:, 1 + c0 : 1 + c0 + ln],
                        scalar=w_pp[:, dt : dt + 1],
                        in1=pbc[:],
                        op0=OP.mult,
                        op1=OP.mult,
                    )

        def load_wblock(dram_mat, ocol, tag):
            """(D, 128) column block of a (D, Dout) DRAM matrix -> bf16
            (P, ND, P) tile; k-tile kt is [:, kt, :]."""
            raw = wpool.tile([P, ND, P], F32, tag="wraw")
            src = dram_mat.rearrange("(kt p) o -> p kt o", p=P)[
                :, :, ocol * P : (ocol + 1) * P
            ]
            nc.sync.dma_start(out=raw[:], in_=src)
            wb = wbpool.tile([P, ND, P], BF16, tag="wb")
            nc.vector.tensor_copy(wb[:], raw[:])
            return wb

        # ================= layers =================
        for l in range(L):
            s0 = SHIFT * l
            chs = _chunks(s0)
            win = FRAME - s0

            cwt = lpool.tile([P, ND, K], F32, tag="cw")
            nc.sync.dma_start(
                out=cwt[:], in_=cw_d[l].rearrange("(dt p) k -> p dt k", p=P)
            )
            cbt = load_bias(cb_d[l], "cb")
            pwbt = load_bias(pwb_d[l], "pwb")
            n1t = load_bias(n1w_d[l], "n1")
            n2t = load_bias(n2w_d[l], "n2")
            zbt = load_bias(zb_d[l], "zb")
            nzbt = lpool.tile([P, ND], F32, tag="nzb")
            nc.vector.tensor_scalar_mul(nzbt[:], zbt[:], -1.0)
            hbt = load_bias(hb_d[l], "hb")
            hbpt = lpool.tile([P, ND], F32, tag="hbp")
            nc.vector.tensor_scalar_add(hbpt[:], hbt[:], 0.5)
            f1bt = load_bias(f1b_d[l], "f1b")
            f2bt = load_bias(f2b_d[l], "f2b")

            # zero the two conv-boundary columns (per-core mask)
            for dt in range(ND):
                for mi, mc in enumerate(MASK_COLS):
                    nc.vector.tensor_tensor(
                        out=hres[dt][:, 1 + mc : 2 + mc],
                        in0=hres[dt][:, 1 + mc : 2 + mc],
                        in1=mask2[:, mi : mi + 1],
                        op=OP.mult,
                    )

            # depthwise conv (bf16 out), chunked
            cvt = [apool.tile([P, win], BF16, name=f"cvt{dt}", tag=f"a{dt}") for dt in range(ND)]
            for dt in range(ND):
                for c0, ln in chs:
                    dst = cvt[dt][:, c0 - s0 : c0 - s0 + ln]
                    nc.scalar.activation(
                        dst, hres[dt][:, c0 : c0 + ln], AF.Identity,
                        bias=cbt[:, dt : dt + 1],
                        scale=cwt[:, dt, 0:1],
                    )
                    nc.vector.scalar_tensor_tensor(
                        out=dst, in0=hres[dt][:, 1 + c0 : 1 + c0 + ln],
                        scalar=cwt[:, dt, 1:2],
                        in1=dst, op0=OP.mult, op1=OP.add,
                    )
                    nc.vector.scalar_tensor_tensor(
                        out=dst, in0=hres[dt][:, 2 + c0 : 2 + c0 + ln],
                        scalar=cwt[:, dt, 2:3],
                        in1=dst, op0=OP.mult, op1=OP.add,
                    )

            # pointwise conv matmul + residual (in-place h update)
            for od in range(ND):
                wb = load_wblock(pwT_d[l], od, "pw")
                for c0, ln in chs:
                    ps = mm.tile([P, ln], F32, tag="ps")
                    for kt in range(ND):
                        nc.tensor.matmul(
                            out=ps[:], lhsT=wb[:, kt, :],
                            rhs=cvt[kt][:, c0 - s0 : c0 - s0 + ln],
                            start=(kt == 0), stop=(kt == ND - 1),
                        )
                    nc.vector.scalar_tensor_tensor(
                        out=hres[od][:, 1 + c0 : 1 + c0 + ln],
                        in0=ps[:], scalar=pwbt[:, od : od + 1],
                        in1=hres[od][:, 1 + c0 : 1 + c0 + ln],
                        op0=OP.add, op1=OP.add,
                    )

            # rmsnorm1 -> xn (bf16, bpool)
            xn = [bpool.tile([P, win], BF16, name=f"xn{dt}", tag=f"b{dt}") for dt in range(ND)]
            rmsnorm_to_bf16(s0, n1t, xn)

            # z / h_tilde matmuls + scan, per output feature tile
            for od in range(ND):
                zwb = load_wblock(zwT_d[l], od, "zw")
                hwb = load_wblock(hwT_d[l], od, "hw")
                prev_g = None
                for c0, ln in chs:
                    psk = mm.tile([P, ln], F32, tag="ps")
                    for kt in range(ND):
                        nc.tensor.matmul(
                            out=psk[:], lhsT=zwb[:, kt, :],
                            rhs=xn[kt][:, c0 - s0 : c0 - s0 + ln],
                            start=(kt == 0), stop=False,
                        )
                    nc.tensor.matmul(
                        out=psk[:], lhsT=neg40[:], rhs=orow_b[:, c0 : c0 + ln],
                        start=False, stop=True,
                    )
                    psh = mm.tile([P, ln], F32, tag="ps")
                    for kt in range(ND):
                        nc.tensor.matmul(
                            out=psh[:], lhsT=hwb[:, kt, :],
                            rhs=xn[kt][:, c0 - s0 : c0 - s0 + ln],
                            start=(kt == 0), stop=(kt == ND - 1),
                        )
                    ct = spool.tile([P, ln], F32, tag="ct", bufs=1)
                    nc.scalar.activation(
                        ct[:], psk[:], AF.Sigmoid,
                        bias=nzbt[:, od : od + 1], scale=-1.0,
                    )
                    sigt = spool.tile([P, ln], F32, tag="sigt", bufs=1)
                    nc.scalar.activation(
                        sigt[:], psk[:], AF.Sigmoid, bias=zbt[:, od : od + 1]
                    )
                    lint = spool.tile([P, ln], F32, tag="lint", bufs=1)
                    nc.scalar.activation(
                        lint[:], psh[:], AF.Identity, bias=hbpt[:, od : od + 1]
                    )
                    sgt = spool.tile([P, ln], F32, tag="sgt", bufs=1)
                    nc.scalar.activation(
                        sgt[:], psh[:], AF.Sigmoid, bias=hbt[:, od : od + 1]
                    )
                    # g = max(lin, sg) in place of lint; v = sig*g in place
                    nc.vector.tensor_tensor(lint[:], lint[:], sgt[:], op=OP.max)
                    nc.vector.tensor_tensor(sigt[:], sigt[:], lint[:], op=OP.mult)
                    gsc = gpool.tile([P, ln], F32, tag="gsc")
                    init = (
                        g_hp[:, od : od + 1]
                        if prev_g is None
                        else prev_g[0][:, prev_g[1] - 1 : prev_g[1]]
                    )
                    nc.vector.tensor_tensor_scan(
                        gsc[:], ct[:], sigt[:], init, op0=OP.mult, op1=OP.add
                    )
                    prev_g = (gsc, ln)
                    nc.vector.tensor_tensor(
                        out=hres[od][:, 1 + c0 : 1 + c0 + ln],
                        in0=hres[od][:, 1 + c0 : 1 + c0 + ln],
                        in1=gsc[:], op=OP.add,
                    )

            # rmsnorm2 -> xn2 (apool; cvt is dead)
            xn2 = [apool.tile([P, win], BF16, name=f"xn2_{dt}", tag=f"a{dt}") for dt in range(ND)]
            rmsnorm_to_bf16(s0, n2t, xn2)

            # FFN
            gel = [bpool.tile([P, win], BF16, name=f"gel{dt}", tag=f"b{dt}") for dt in range(ND)]
            for oh in range(ND):
                wb = load_wblock(f1w_d[l], oh, "f1")
                for c0, ln in chs:
                    ps = mm.tile([P, ln], F32, tag="ps")
                    for kt in range(ND):
                        nc.tensor.matmul(
                            out=ps[:], lhsT=wb[:, kt, :],
                            rhs=xn2[kt][:, c0 - s0 : c0 - s0 + ln],
                            start=(kt == 0), stop=(kt == ND - 1),
                        )
                    nc.scalar.activation(
                        gel[oh][:, c0 - s0 : c0 - s0 + ln], ps[:], AF.Gelu,
                        bias=f1bt[:, oh : oh + 1],
                    )
            for od in range(ND):
                wb = load_wblock(f2w_d[l], od, "f2")
                for c0, ln in chs:
                    ps = mm.tile([P, ln], F32, tag="ps")
                    for kt in range(ND):
                        nc.tensor.matmul(
                            out=ps[:], lhsT=wb[:, kt, :],
                            rhs=gel[kt][:, c0 - s0 : c0 - s0 + ln],
                            start=(kt == 0), stop=(kt == ND - 1),
                        )
                    nc.vector.scalar_tensor_tensor(
                        out=hres[od][:, 1 + c0 : 1 + c0 + ln],
                        in0=ps[:], scalar=f2bt[:, od : od + 1],
                        in1=hres[od][:, 1 + c0 : 1 + c0 + ln],
                        op0=OP.add, op1=OP.add,
                    )

        # ================= final norm + output =================
        nft = load_bias(nfw_d, "nf")
        xnf = [apool.tile([P, VALID], BF16, name=f"xnf{dt}", tag=f"a{dt}") for dt in range(ND)]
        for ci in range(VALID // CHUNK):
            c0 = LHALO + ci * CHUNK
            pss = ssp.tile([1, CHUNK], F32, tag="pss")
            for dt in range(ND):
                sq = spool.tile([P, CHUNK], BF16, tag="sqc")
                nc.scalar.activation(
                    sq[:], hres[dt][:, 1 + c0 : 1 + c0 + CHUNK], AF.Square
                )
                nc.tensor.matmul(
                    out=pss[:], lhsT=ones_col[:], rhs=sq[:],
                    start=(dt == 0), stop=(dt == ND - 1),
                )
            srow = spool.tile([1, CHUNK], F32, tag="srow", bufs=1)
            nc.vector.tensor_scalar(
                srow[:], pss[:], 1.0 / D, EPS, op0=OP.mult, op1=OP.add
            )
            rt = spool.tile([1, CHUNK], F32, tag="rt", bufs=1)
            nc.scalar.activation(rt[:], srow[:], AF.Sqrt)
            nc.vector.reciprocal(srow[:], rt[:])
            pbc = mm.tile([P, CHUNK], F32, tag="ps")
            nc.tensor.matmul(out=pbc[:], lhsT=one_row[:], rhs=srow[:],
                             start=True, stop=True)
            for dt in range(ND):
                nc.vector.scalar_tensor_tensor(
                    out=xnf[dt][:, ci * CHUNK : (ci + 1) * CHUNK],
                    in0=hres[dt][:, 1 + c0 : 1 + c0 + CHUNK],
                    scalar=nft[:, dt : dt + 1],
                    in1=pbc[:], op0=OP.mult, op1=OP.mult,
                )

        # out matmul, token-major output
        outw_b = []
        for kt in range(ND):
            raw = wpool.tile([P, H], F32, tag="wraw")
            nc.sync.dma_start(out=raw[:], in_=outw_d[kt * P : (kt + 1) * P, :])
            wb = bpool.tile([P, H], BF16, tag=f"b{kt}")  # gel slots are dead
            nc.vector.tensor_copy(wb[:], raw[:])
            outw_b.append(wb)
        for ttv in range(VALID // P):
            for oc in range(H // CHUNK):
                ps = mm.tile([P, CHUNK], F32, tag="ps")
                for kt in range(ND):
                    nc.tensor.matmul(
                        out=ps[:],
                        lhsT=xnf[kt][:, ttv * P : (ttv + 1) * P],
                        rhs=outw_b[kt][:, oc * CHUNK : (oc + 1) * CHUNK],
                        start=(kt == 0), stop=(kt == ND - 1),
                    )
                osb = spool.tile([P, CHUNK], F32, tag="osb")
                nc.vector.tensor_tensor(
                    osb[:], ps[:], ob_bc[:, oc * CHUNK : (oc + 1) * CHUNK],
                    op=OP.add,
                )
                nc.sync.dma_start(
                    out=y_d[ttv * P : (ttv + 1) * P, oc * CHUNK : (oc + 1) * CHUNK],
                    in_=osb[:],
                )

    nc.finalize()
    cap_sync_waits(nc, cap=1)
    return nc


_nc_cache = [None]


def get_nc():
    if _nc_cache[0] is None:
        _nc_cache[0] = build_program()
    return _nc_cache[0]


def make_in_maps(inputs):
    x = np.asarray(inputs["x"]).astype(np.int64)
    h_prev = np.asarray(inputs["h_prev"], dtype=np.float32)
    f32 = lambda k: np.ascontiguousarray(np.asarray(inputs[k], dtype=np.float32))
    common = {
        "emb": f32("emb"),
        "cw": f32("conv_dw_w").reshape(L, D, K),
        "cb": f32("conv_dw_b"),
        "pwT": np.ascontiguousarray(f32("conv_pw_w").transpose(0, 2, 1)),
        "pwb": f32("conv_pw_b"),
        "n1w": f32("norm1_w"),
        "n2w": f32("norm2_w"),
        "zwT": np.ascontiguousarray(f32("zw").transpose(0, 2, 1)),
        "zb": f32("zb"),
        "hwT": np.ascontiguousarray(f32("hw").transpose(0, 2, 1)),
        "hb": f32("hb"),
        "f1w": f32("f1w"),
        "f1b": f32("f1b"),
        "f2w": f32("f2w"),
        "f2b": f32("f2b"),
        "nfw": f32("normf_w"),
        "outw": f32("out_w"),
        "outb": f32("out_b"),
    }
    in_maps = []
    for core in range(8):
        b, half = core // 2, core % 2
        start = half * VALID - LHALO
        gidx = np.arange(start, start + FRAME)
        inseq = (gidx >= 0) & (gidx < S)
        ids = np.where(inseq, x[b, np.clip(gidx, 0, S - 1)], 0).astype(np.int32)
        mask = inseq.astype(np.float32)
        m = dict(common)
        m["ids"] = ids
        m["m2"] = np.ascontiguousarray(
            mask[list(MASK_COLS)][None, :].astype(np.float32)
        )
        import ml_dtypes
        m["omm"] = np.ascontiguousarray(
            (1.0 - mask)[None, :].astype(ml_dtypes.bfloat16)
        )
        m["hprev"] = np.ascontiguousarray(h_prev[b, 0])
        in_maps.append(m)
    return in_maps


def kernel(**inputs):
    from concourse.bass_utils import run_bass_kernel_spmd

    nc = get_nc()
    in_maps = make_in_maps(inputs)
    res = run_bass_kernel_spmd(nc, in_maps, list(range(8)))
    out = np.zeros((B, S, H), np.float32)
    for core in range(8):
        b, half = core // 2, core % 2
        out[b, half * VALID : (half + 1) * VALID] = res.results[core]["y"]
    return out
